# revision 30
# baseline (speedup 1.0000x reference)
"""MoE-SIREN (nn_MoE_36146444763329) Trainium2 Bass kernel, v3: N/D tables.

The whole MoE output is a univariate function of x:
    f(x) = N(x)/D(x),  N = sum_e exp(z_e(x)) y_e(x),  D = sum_e exp(z_e(x)),
with z_e = gate logits. Each core (range-sharded points, 64 global bins):
  1. builds the 8 expert SIREN tables over ITS x-range on device (fp16
     matmuls, fp32 range-reduction, ACT Sin; Sin is the ONLY Act function
     so its table set loads once at t=0),
  2. evaluates the gate exp on the sample grid with a Chebyshev polynomial
     on DVE/Pool (no Act Exp -> no second act-table load),
  3. forms the transposed tables y^T[sample, expert] directly via PE
     matmuls with h3 as the stationary operand (Ldweights is free),
  4. reduces N/D over experts (DVE free-dim reduce) and turns the tables
     into per-bin Catmull-Rom coefficients with host-baked shift x CR
     matrices as PE matmuls -> coefficients land as per-partition scalars,
  5. evaluates two cubic Horner chains + one divide per point.

No per-point exp, no moment products, no mid-kernel SBUF->SBUF coef DMA
(each DMA costs ~2.2us fixed latency in HWDGE+DGE+sem-prop). Weight DMAs
are chunked so each layer's matmuls start as soon as its bytes land.

Points layout per core: partition p = bin + 64*half, C2 slots per half
(C2 grown if an input overflows; seed-0 max occupancy is 86 -> C2=44).
Host binning is index computation only; all value math runs on device.
"""

import numpy as np

import concourse.bass as bass
import concourse.mybir as mybir
import concourse.tile as tile
from concourse import bacc
from concourse.bass_utils import run_bass_kernel_spmd
from concourse.dve_ops import ADD_RANGE_WRAP

F32 = mybir.dt.float32
F16 = mybir.dt.float16
AT = mybir.ActivationFunctionType
ALU = mybir.AluOpType
AXL = mybir.AxisListType

B, N, E, H, NLAYERS = 2, 16384, 8, 256, 4
OMEGA0 = 30.0
NCORES = 8
NHID = NLAYERS - 1
TWO_PI = float(2.0 * np.pi)
SC = float(OMEGA0 / (2.0 * np.pi))
MAGIC = float(np.float32(1.5 * 2 ** 23))

M = 256                      # global bins
BPC = M // NCORES            # 32 bins per core
NS = 36                      # build sample batch (35 used: bins+3, padded)
NSU = 35                     # used samples
# PSUM bank = 512 fp32/partition and a matmul output may not cross a bank
# boundary: place 14 NS-wide units per 512-col bank (pad 8 cols), 16 units
# -> 2 banks, so each hidden layer gets its own PSUM bank pair (no reuse).
SLAB = 2 * 512


def ucol(u):
    return 512 * (u // 14) + NS * (u % 14)


# elementwise spans covering exactly the written unit regions (pads excluded).
# The first span holds only units 0,1 (= expert 0) so the next layer's first
# matmuls unblock after a small wrap+sin, not a 476-col one.
SPANS = [(0, 2 * NS), (2 * NS, 7 * NS), (7 * NS, 14 * NS),
         (512, 512 + 2 * NS)]
SPAN_UNITS = [(0, 2), (2, 7), (7, 14), (14, 16)]

# consts tile [128, 64] column layout
C_A0 = 0      # 16: layer-0 scale per unit u=e*2+half
C_C0 = 16     # 16: layer-0 bias
C_BIN = 32    # 1: global bin index per partition (j*32 + p//4, bin-major)
C_XS = 33     # 1: sample-grid position per partition (rows 0..66)
C_GW = 34     # 8: gate_w replicated over sample rows
C_GB = 42     # 8: gate_b replicated
C_BO = 50     # 8: output bias replicated over sample rows
# cf32 = cst(64) | xs(NS) | xb(C2) | crx(4*128 f32, rows 0..66)
CST_W = 64

_BUILD_CACHE: dict = {}
LAST_RESULT = None
DEBUG = False


def _cheb_exp(R):
    """Chebyshev fit of exp on [-R, 0]; ascending coeffs + max abs err."""
    from numpy.polynomial import chebyshev as _C
    from numpy.polynomial import polynomial as _P
    for deg in range(6, 15):
        ch = _C.Chebyshev.interpolate(np.exp, deg, domain=[-R, 0.0])
        p = ch.convert(kind=_P.Polynomial)
        zz = np.linspace(-R, 0, 4001)
        err = float(np.abs(p(zz) - np.exp(zz)).max())
        if err < 5e-5:
            return [float(c) for c in p.coef], err
    return [float(c) for c in p.coef], err


def _build(wrap_twice: bool, C2: int, R: float):
    cheb, _ = _cheb_exp(R)
    deg = len(cheb) - 1
    nc = bacc.Bacc("TRN2", target_bir_lowering=False, debug=False,
                   num_devices=NCORES)

    CW = CST_W + NS + C2
    d_cf32 = nc.dram_tensor("cf32", [128, CW + 512], F32, kind="ExternalInput")
    d_r16 = nc.dram_tensor("r16", [1, NHID * 16 * 128 + 8], F16,
                           kind="ExternalInput")
    d_w16 = nc.dram_tensor("w16", [128, NHID * 4096 + 16], F16,
                           kind="ExternalInput")
    d_out = nc.dram_tensor("out", [128, C2], F32, kind="ExternalOutput")

    with tile.TileContext(nc) as tc:
        with (
            tc.tile_pool(name="cst", bufs=1) as cst_pool,
            tc.tile_pool(name="whp", bufs=1) as wh_pool,
            tc.tile_pool(name="bld", bufs=1) as b_pool,
            tc.tile_pool(name="vwr", bufs=1) as v_pool,
            tc.tile_pool(name="evl", bufs=1) as e_pool,
            tc.tile_pool(name="zpsa", bufs=1, space="PSUM") as z_ps_a,
            tc.tile_pool(name="zpsb", bufs=1, space="PSUM") as z_ps_b,
            tc.tile_pool(name="zpsc", bufs=1, space="PSUM") as z_ps_c,
            tc.tile_pool(name="yps", bufs=1, space="PSUM") as y_ps,
            tc.tile_pool(name="cps", bufs=1, space="PSUM") as c_ps,
        ):
            # ---- input DMAs spread across HWDGE issue queues (SP/Act/DVE)
            # + Pool SWDGE so descriptor issue (650ns each, serial per
            # queue) never gates the copy chain; descriptor-ready order =
            # desired copy order on the shared DMA engines.
            t_cf = cst_pool.tile([128, CW], F32, tag="cf32")
            t_r16 = cst_pool.tile([1, NHID * 16 * 128 + 8], F16, tag="r16")
            t_wh = [wh_pool.tile([128, 4096], F16, tag=f"wh{l}", name=f"wh{l}")
                    for l in range(NHID)]
            t_wo = wh_pool.tile([128, 16], F16, tag="wo")
            t_crx = cst_pool.tile([128, 512], F32, tag="crx")

            def wh_chunk(l, a, b):
                return (t_wh[l][:, a:b], d_w16[:, l * 4096 + a:l * 4096 + b])

            # chunks are expert-aligned (e0-3 | e4-6 | e7); queue slots are
            # chosen so each chunk's descriptor-ready time (SP: 666+650k,
            # Act: 666+657k+134) lands in desired copy order on the FIFO
            # DMA engines.
            # Act gets exactly ONE issue so the Sin act-table load (also on
            # the Act queue) still lands by ~2.6us.
            nc.sync.dma_start(t_cf[:], d_cf32[:, 0:CW])          # SP#1
            nc.scalar.dma_start(*wh_chunk(0, 0, 2048))           # Act#1
            nc.gpsimd.dma_start(t_r16[:], d_r16[:, :])           # Pool#1
            nc.sync.dma_start(*wh_chunk(0, 2048, 3584))          # SP#2
            nc.sync.dma_start(*wh_chunk(1, 0, 2048))             # SP#3
            nc.gpsimd.dma_start(*wh_chunk(0, 3584, 4096))        # Pool#2
            nc.sync.dma_start(*wh_chunk(1, 2048, 3584))          # SP#4
            nc.sync.dma_start(*wh_chunk(1, 3584, 4096))          # SP#5
            nc.sync.dma_start(*wh_chunk(2, 0, 2048))             # SP#6
            nc.sync.dma_start(*wh_chunk(2, 2048, 3584))          # SP#7
            nc.sync.dma_start(*wh_chunk(2, 3584, 4096))          # SP#8
            nc.sync.dma_start(t_wo[:],
                              d_w16[:, NHID * 4096:NHID * 4096 + 16])
            nc.sync.dma_start(t_crx[0:NSU, :], d_cf32[0:NSU, CW:CW + 512])

            t_cst = t_cf[:, 0:CST_W]
            t_xs = t_cf[:, CST_W:CST_W + NS]
            t_xb = t_cf[:, CST_W + NS:CST_W + NS + C2]
            ap_bin = t_cf[:, C_BIN:C_BIN + 1]
            ap_xsc = t_cf[0:NSU, C_XS:C_XS + 1]
            ap_gw = t_cf[0:NSU, C_GW:C_GW + 8]
            ap_gb = t_cf[0:NSU, C_GB:C_GB + 8]
            ap_bo = t_cf[0:NSU, C_BO:C_BO + 8]

            t_on = cst_pool.tile([1, NS], F16, tag="ones")
            nc.gpsimd.memset(t_on[:], 1.0)

            # ---- layer 0 (zb = a*x + c; magic round; ACT Sin).
            # Sin is the only Act function -> its table set loads at t~0.
            # All SBUF staging tiles are PER SPAN: subtile writes into one
            # big tile coarsen to whole-tile WAW deps, which the sem
            # splitter turns into SEQ-blocking EventSemaphores.
            SPAN_W = [hi - lo for lo, hi in SPANS]

            def uspan(u):
                return 0 if u < 2 else 1 if u < 7 else 2 if u < 14 else 3

            def ucs(u):
                return ucol(u) - SPANS[uspan(u)][0]

            t_zb = [b_pool.tile([128, SPAN_W[si]], F32, tag=f"zb{si}",
                                 name=f"zb{si}") for si in range(4)]
            t_k = [b_pool.tile([128, SPAN_W[si]], F32, tag=f"k{si}",
                                name=f"k{si}") for si in range(4)]
            t_v0 = [b_pool.tile([128, SPAN_W[si]], F32, tag=f"v0{si}",
                                 name=f"v0{si}") for si in range(4)]
            t_h = [[b_pool.tile([128, SPAN_W[si]], F16, tag=f"h{l}_{si}",
                                 name=f"h{l}_{si}") for si in range(4)]
                   for l in range(NLAYERS)]

            def zb_unit(u):
                eng = nc.vector if u % 2 == 0 else nc.gpsimd
                eng.tensor_scalar(t_zb[uspan(u)][:, ucs(u):ucs(u) + NS], t_xs,
                                  t_cf[:, C_A0 + u:C_A0 + u + 1],
                                  t_cf[:, C_C0 + u:C_C0 + u + 1],
                                  ALU.mult, ALU.add)

            def l0_span(si):
                nc.vector.tensor_scalar(t_k[si][:], t_zb[si][:], MAGIC, MAGIC,
                                        ALU.add, ALU.subtract)
                nc.vector.tensor_tensor(t_v0[si][:], t_zb[si][:], t_k[si][:],
                                        ALU.subtract)
                nc.scalar.activation(t_h[0][si][:], t_v0[si][:], AT.Sin,
                                     bias=0.0, scale=TWO_PI)

            # span-ordered: units of span si then its round+sin; the gate-exp
            # chain (tiny [67,8] ops) slots in after span 1 so it runs in the
            # weight-DMA shadow without delaying the first sins.
            for si in range(4):
                for u in range(*SPAN_UNITS[si]):
                    zb_unit(u)
                l0_span(si)
                if si == 0:
                    t_z = e_pool.tile([NSU, 8], F32, tag="z")
                    t_s = e_pool.tile([NSU, 1], F32, tag="s")
                    t_z2 = e_pool.tile([NSU, 8], F32, tag="z2")
                    nc.vector.scalar_tensor_tensor(t_z[:], ap_gw, ap_xsc,
                                                   ap_gb, ALU.mult, ALU.add)
                    nc.vector.tensor_reduce(t_s[:], t_z[:], AXL.X, ALU.max)
                    nc.vector.tensor_scalar(t_z2[:], t_z[:], t_s[:],
                                            -float(R), ALU.subtract, ALU.max)
                    # Horner on Pool (gpsimd): u = cheb(z2), abs err < 5e-5
                    t_u = e_pool.tile([NSU, 8], F32, tag="u")
                    t_hz = e_pool.tile([NSU, 8], F32, tag="hz")
                    nc.gpsimd.tensor_scalar(t_u[:], t_z2[:], cheb[deg],
                                            cheb[deg - 1], ALU.mult, ALU.add)
                    for k in range(deg - 2, -1, -1):
                        nc.gpsimd.tensor_tensor(t_hz[:], t_u[:], t_z2[:],
                                                ALU.mult)
                        nc.gpsimd.tensor_scalar_add(t_u[:], t_hz[:], cheb[k])
                    # D table: expert tree-sum of u, kept on Pool so it
                    # queues behind the Horner instead of stalling DVE.
                    t_nd = e_pool.tile([NSU, 2], F32, tag="nd")
                    t_a4 = e_pool.tile([NSU, 4], F32, tag="a4")
                    nc.gpsimd.tensor_tensor(t_a4[:], t_u[:, 0:4],
                                            t_u[:, 4:8], ALU.add)
                    nc.gpsimd.tensor_tensor(t_a4[:, 0:2], t_a4[:, 0:2],
                                            t_a4[:, 2:4], ALU.add)
                    nc.gpsimd.tensor_tensor(t_nd[:, 1:2], t_a4[:, 0:1],
                                            t_a4[:, 1:2], ALU.add)

            # ---- per-point t, t^2 (off critical path, DMA shadow)
            t_t = e_pool.tile([128, C2], F32, tag="t")
            t_t2 = e_pool.tile([128, C2], F32, tag="t2")
            nc.vector.tensor_scalar(t_t[:], t_xb, float(M), ap_bin,
                                    ALU.mult, ALU.subtract)
            nc.gpsimd.tensor_tensor(t_t2[:], t_t[:], t_t[:], ALU.mult)

            # ---- hidden layers: fp16 matmuls (+fp16 K=1 bias matmul),
            # range-reduce on DVE, Sin on Act
            for l in range(1, NLAYERS):
                lw = l - 1
                p_z = (z_ps_a, z_ps_b, z_ps_c)[l - 1].tile(
                    [128, SLAB], F32, tag="z", name=f"z{l}")
                for si in range(4):
                    for u in range(*SPAN_UNITS[si]):
                        e, half = divmod(u, 2)
                        out_sl = p_z[:, ucol(u):ucol(u) + NS]
                        for kc in range(2):
                            wc = ((e * 2 + kc) * 2 + half) * 128
                            uu = e * 2 + kc
                            nc.tensor.matmul(
                                out_sl, t_wh[lw][:, wc:wc + 128],
                                t_h[l - 1][uspan(uu)][:, ucs(uu):
                                                      ucs(uu) + NS],
                                start=(kc == 0), stop=False)
                        bc = (lw * 16 + u) * 128
                        nc.tensor.matmul(out_sl, t_r16[0:1, bc:bc + 128],
                                         t_on[:], start=False, stop=True)
                    lo, hi = SPANS[si]
                    sl = slice(lo, hi)
                    t_v = v_pool.tile([128, SPAN_W[si]], F32,
                                      tag=f"v{l}_{si}", name=f"v{l}_{si}")
                    if wrap_twice:
                        t_w2 = v_pool.tile([128, SPAN_W[si]], F32,
                                           tag=f"w2_{l}{si}",
                                           name=f"w2_{l}{si}")
                        nc.vector._custom_dve(ADD_RANGE_WRAP, out=t_w2[:],
                                              in0=p_z[:, sl], s0=0.0,
                                              s1=1.0, imm2=2.0)
                        nc.vector._custom_dve(ADD_RANGE_WRAP, out=t_v[:],
                                              in0=t_w2[:], s0=0.0,
                                              s1=0.5, imm2=1.0)
                    else:
                        nc.vector._custom_dve(ADD_RANGE_WRAP, out=t_v[:],
                                              in0=p_z[:, sl], s0=0.0,
                                              s1=0.5, imm2=1.0)
                    nc.scalar.activation(t_h[l][si][:], t_v[:], AT.Sin,
                                         bias=0.0, scale=TWO_PI)

            # ---- output layer, transposed: p_yT[s, e] via h3-stationary
            # matmuls (Ldweights is free); +bo via a K=1 ones matmul.
            p_yT = y_ps.tile([NSU, 8], F32, tag="yT")
            for e in range(E):
                for kc in range(2):
                    u = e * 2 + kc
                    nc.tensor.matmul(
                        p_yT[:, e:e + 1],
                        t_h[NLAYERS - 1][uspan(u)][:, ucs(u):ucs(u) + NSU],
                        t_wo[:, u:u + 1],
                        start=(kc == 0), stop=(kc == 1),
                        skip_group_check=True)

            # ---- N table + Catmull-Rom coefficients via PE
            # (bo is folded in on DVE: et = (yT + bo) * u)
            t_et = e_pool.tile([NSU, 8], F32, tag="et")
            t_yb = e_pool.tile([NSU, 8], F32, tag="yb")
            nc.vector.tensor_tensor(t_yb[:], p_yT[:], ap_bo, ALU.add)
            nc.vector.scalar_tensor_tensor(t_et[:], t_yb[:], 1.0, t_u[:],
                                           ALU.mult, ALU.mult,
                                           accum_out=t_nd[:, 0:1])
            p_coef = c_ps.tile([128, 8], F32, tag="coef")
            for X in range(4):
                for f in range(2):
                    nc.tensor.matmul(p_coef[:, X + 4 * f:X + 4 * f + 1],
                                     t_crx[0:NSU, X * 128:(X + 1) * 128],
                                     t_nd[:, f:f + 1], start=True, stop=True)

            # ---- per-point cubics: NH = (cA t + cB) t^2 + (cC t + cD),
            # same for DH; out = NH/DH. All on DVE (coef scalars straight
            # from PSUM; GPSIMD can't read PSUM).
            cN = [p_coef[:, i:i + 1] for i in range(4)]
            cD = [p_coef[:, 4 + i:5 + i] for i in range(4)]
            t_w1 = e_pool.tile([128, C2], F32, tag="w1")
            t_w2e = e_pool.tile([128, C2], F32, tag="w2e")
            t_w3 = e_pool.tile([128, C2], F32, tag="w3")
            t_vD = e_pool.tile([128, C2], F32, tag="vD")
            t_u1 = e_pool.tile([128, C2], F32, tag="u1")
            t_u2 = e_pool.tile([128, C2], F32, tag="u2")
            t_u3 = e_pool.tile([128, C2], F32, tag="u3")
            t_vN = e_pool.tile([128, C2], F32, tag="vN")
            t_rcp = e_pool.tile([128, C2], F32, tag="rcp")
            t_outm = e_pool.tile([128, C2], F32, tag="outm")
            nc.vector.tensor_scalar(t_w1[:], t_t[:], cD[0], cD[1],
                                    ALU.mult, ALU.add)
            nc.vector.tensor_scalar(t_w2e[:], t_t[:], cD[2], cD[3],
                                    ALU.mult, ALU.add)
            nc.vector.tensor_tensor(t_w3[:], t_w1[:], t_t2[:], ALU.mult)
            nc.vector.tensor_tensor(t_vD[:], t_w3[:], t_w2e[:], ALU.add)
            nc.vector.reciprocal(t_rcp[:], t_vD[:])
            nc.vector.tensor_scalar(t_u1[:], t_t[:], cN[0], cN[1],
                                    ALU.mult, ALU.add)
            nc.vector.tensor_scalar(t_u2[:], t_t[:], cN[2], cN[3],
                                    ALU.mult, ALU.add)
            nc.gpsimd.tensor_tensor(t_u3[:], t_u1[:], t_t2[:], ALU.mult)
            nc.vector.tensor_tensor(t_vN[:], t_u3[:], t_u2[:], ALU.add)
            nc.vector.tensor_tensor(t_outm[:], t_vN[:], t_rcp[:], ALU.mult)
            nc.sync.dma_start(d_out[:, :], t_outm[:])
            if DEBUG:
                d_dbg = nc.dram_tensor("dbg", [128, 48], F32,
                                       kind="ExternalOutput")
                nc.sync.dma_start(d_dbg[0:67, 0:8], t_u[:])
                nc.sync.dma_start(d_dbg[0:67, 8:10], t_nd[:])
                nc.sync.dma_start(d_dbg[:, 16:24], t_coef[:])
                nc.sync.dma_start(d_dbg[:, 24:32], t_t[:, 0:8])
                nc.sync.dma_start(d_dbg[0:67, 32:40], t_et[:])
                nc.sync.dma_start(d_dbg[0:67, 40:48], t_z[:])
                d_yT = nc.dram_tensor("dbg_yT", [67, 8], F32,
                                      kind="ExternalOutput")
                t_yTc = e_pool.tile([67, 8], F32, tag="yTc")
                nc.vector.tensor_scalar(t_yTc[:], p_yT[:], 1.0, None,
                                        ALU.mult)
                nc.sync.dma_start(d_yT[:, :], t_yTc[:])

    nc.compile()
    return nc


def _host_pack(x, gate_w, gate_b, w0, b0, wh, bh, wo, bo, C2):
    """Host: range-shard + bin points, pack weights/consts. Index math only."""
    xf = x.reshape(-1)
    NP = xf.size
    gbin = np.clip((xf.astype(np.float64) * M).astype(np.int64), 0, M - 1)
    core = gbin >> 5
    binlo = gbin & (BPC - 1)

    counts = np.zeros((NCORES, BPC), np.int64)
    slot = np.empty(NP, np.int64)
    for i in range(NP):
        c, b = core[i], binlo[i]
        slot[i] = counts[c, b]
        counts[c, b] += 1
    maxc = int(counts.max())
    if maxc > 4 * C2:
        return None, None, maxc

    placement = (core, binlo, slot)

    # fp16 weight pack (shared across cores): wh blocks | wo columns
    w16 = np.zeros((128, NHID * 4096 + 16), np.float16)
    for l in range(NHID):
        for e in range(E):
            for kc in range(2):
                for mc in range(2):
                    colbase = l * 4096 + ((e * 2 + kc) * 2 + mc) * 128
                    blk = (SC * wh[l, e, mc * 128:(mc + 1) * 128,
                                   kc * 128:(kc + 1) * 128]).T
                    w16[:, colbase:colbase + 128] = blk.astype(np.float16)
    for u in range(16):
        e, half = divmod(u, 2)
        w16[:, NHID * 4096 + u] = \
            wo[e, 0, half * 128:(half + 1) * 128].astype(np.float16)

    r16 = np.zeros((1, NHID * 16 * 128 + 8), np.float16)
    for l in range(NHID):
        for u in range(16):
            e, half = divmod(u, 2)
            r16[0, (l * 16 + u) * 128:(l * 16 + u + 1) * 128] = \
                (SC * bh[l, e, half * 128:(half + 1) * 128]).astype(np.float16)
    r16[0, NHID * 16 * 128:] = bo[:, 0].astype(np.float16)

    # Catmull-Rom shift matrices, f32: crx[s, X*128+p], bin-major
    # partitions p = 4*bin + q; nonzero at s=b..b+3.
    CRW = np.array([[-0.5, 1.5, -1.5, 0.5],
                    [1.0, -2.5, 2.0, -0.5],
                    [-0.5, 0.0, 0.5, 0.0],
                    [0.0, 1.0, 0.0, 0.0]], np.float64)
    crx = np.zeros((128, 512), np.float32)
    for p in range(128):
        b = p // 4
        for X in range(4):
            for k in range(4):
                crx[b + k, X * 128 + p] = CRW[X, k]

    cst_base = np.zeros((128, CST_W), np.float32)
    for u in range(16):
        e, half = divmod(u, 2)
        cst_base[:, C_A0 + u] = SC * w0[e, half * 128:(half + 1) * 128, 0]
        cst_base[:, C_C0 + u] = SC * b0[e, half * 128:(half + 1) * 128]
    cst_base[0:NSU, C_GW:C_GW + 8] = gate_w[:, 0][None, :]
    cst_base[0:NSU, C_GB:C_GB + 8] = gate_b[None, :]
    cst_base[0:NSU, C_BO:C_BO + 8] = bo[:, 0][None, :]

    CW = CST_W + NS + C2
    in_maps = []
    for j in range(NCORES):
        cst = cst_base.copy()
        cst[:, C_BIN] = j * BPC + (np.arange(128) >> 2)
        gi = j * BPC - 1 + np.arange(NSU)
        cst[0:NSU, C_XS] = (gi / M).astype(np.float32)
        # binned x: partition p = 4*bin + q, C2 slots per quarter
        xbj = np.empty((128, C2), np.float32)
        for b in range(BPC):
            fill = np.full(4 * C2, (j * BPC + b + 0.5) / M, np.float32)
            sel = (core == j) & (binlo == b)
            vals = xf[sel]
            fill[:vals.size] = vals
            for q in range(4):
                xbj[4 * b + q, :] = fill[q * C2:(q + 1) * C2]
        xsg = j * BPC - 1 + np.minimum(np.arange(NS), BPC + 2)
        assert NSU == BPC + 3
        cf32 = np.empty((128, CW + 512), np.float32)
        cf32[:, 0:CST_W] = cst
        cf32[:, CST_W:CST_W + NS] = \
            np.broadcast_to((xsg / M).astype(np.float32), (128, NS))
        cf32[:, CST_W + NS:CW] = xbj
        cf32[:, CW:] = crx
        in_maps.append({"cf32": cf32, "r16": r16, "w16": w16})
    return in_maps, placement, maxc


def kernel(x, gate_w, gate_b, w0, b0, wh, bh, wo, bo):
    x = np.asarray(x, dtype=np.float32)
    gate_w = np.asarray(gate_w, dtype=np.float32)
    gate_b = np.asarray(gate_b, dtype=np.float32)
    w0 = np.asarray(w0, dtype=np.float32)
    b0 = np.asarray(b0, dtype=np.float32)
    wh = np.asarray(wh, dtype=np.float32)
    bh = np.asarray(bh, dtype=np.float32)
    wo = np.asarray(wo, dtype=np.float32)
    bo = np.asarray(bo, dtype=np.float32)

    # hidden pre-activation range (turns) over the union of build grids
    gi = np.arange(-1, M + NS - BPC + 2, dtype=np.float64) / M
    xs = gi.astype(np.float32)
    a = (SC * w0[:, :, 0]).astype(np.float32)
    c = (SC * b0).astype(np.float32)
    zb = a[:, :, None] * xs[None, None, :] + c[:, :, None]
    h = np.sin(TWO_PI * (zb - np.round(zb))).astype(np.float32)
    hid_bound = 0.0
    for l in range(NHID):
        z = (np.einsum('egh,eht->egt', (SC * wh[l]).astype(np.float32), h)
             + (SC * bh[l]).astype(np.float32)[:, :, None])
        hid_bound = max(hid_bound, float(np.abs(z).max()))
        h = np.sin(TWO_PI * (z - np.round(z))).astype(np.float32)
    hid_bound *= 1.02
    assert hid_bound < 2.90, f"hidden range {hid_bound} needs >2 wraps"
    wrap_twice = bool(hid_bound >= 1.45)

    # gate logit spread on the grid -> exp poly range R
    zg = gate_w[:, 0][None, :] * xs[:, None] + gate_b[None, :]
    spread = float((zg.max(1) - zg.min(1)).max())
    R = float(max(2.0, np.ceil(spread + 0.3)))

    C2 = 44
    while True:
        in_maps, placement, maxc = _host_pack(
            x, gate_w, gate_b, w0, b0, wh, bh, wo, bo, C2)
        if in_maps is not None:
            break
        C2 = ((maxc + 3) // 4 + 3) // 4 * 4

    key = (wrap_twice, C2, R)
    if key not in _BUILD_CACHE:
        _BUILD_CACHE[key] = _build(wrap_twice, C2, R)
    nc = _BUILD_CACHE[key]

    global LAST_RESULT
    LAST_RESULT = run_bass_kernel_spmd(nc, in_maps, list(range(NCORES)))
    res = LAST_RESULT.results

    core, binlo, slot = placement
    out = np.empty(x.size, np.float32)
    for j in range(NCORES):
        oj = res[j]["out"]            # [128, C2]
        sel = np.nonzero(core == j)[0]
        sl = slot[sel]
        out[sel] = oj[4 * binlo[sel] + sl // C2, sl % C2]
    return out.reshape(B, N, 1).astype(np.float32)


# revision 31
# speedup vs baseline: 1.1626x; 1.1626x over previous
"""MoE-SIREN (nn_MoE_36146444763329) Trainium2 Bass kernel, v3: N/D tables.

The whole MoE output is a univariate function of x:
    f(x) = N(x)/D(x),  N = sum_e exp(z_e(x)) y_e(x),  D = sum_e exp(z_e(x)),
with z_e = gate logits. Each core (range-sharded points, 64 global bins):
  1. builds the 8 expert SIREN tables over ITS x-range on device (fp16
     matmuls, fp32 range-reduction, ACT Sin; Sin is the ONLY Act function
     so its table set loads once at t=0),
  2. evaluates the gate exp on the sample grid with a Chebyshev polynomial
     on DVE/Pool (no Act Exp -> no second act-table load),
  3. forms the transposed tables y^T[sample, expert] directly via PE
     matmuls with h3 as the stationary operand (Ldweights is free),
  4. reduces N/D over experts (DVE free-dim reduce) and turns the tables
     into per-bin Catmull-Rom coefficients with host-baked shift x CR
     matrices as PE matmuls -> coefficients land as per-partition scalars,
  5. evaluates two cubic Horner chains + one divide per point.

No per-point exp, no moment products, no mid-kernel SBUF->SBUF coef DMA
(each DMA costs ~2.2us fixed latency in HWDGE+DGE+sem-prop). Weight DMAs
are chunked so each layer's matmuls start as soon as its bytes land.

Points layout per core: partition p = bin + 64*half, C2 slots per half
(C2 grown if an input overflows; seed-0 max occupancy is 86 -> C2=44).
Host binning is index computation only; all value math runs on device.
"""

import numpy as np

import concourse.bass as bass
import concourse.mybir as mybir
import concourse.tile as tile
from concourse import bacc
from concourse.bass_utils import run_bass_kernel_spmd
from concourse.dve_ops import ADD_RANGE_WRAP
from concourse.tile_rust import add_dep_helper

F32 = mybir.dt.float32
F16 = mybir.dt.float16
AT = mybir.ActivationFunctionType
ALU = mybir.AluOpType
AXL = mybir.AxisListType

B, N, E, H, NLAYERS = 2, 16384, 8, 256, 4
OMEGA0 = 30.0
NCORES = 8
NHID = NLAYERS - 1
TWO_PI = float(2.0 * np.pi)
SC = float(OMEGA0 / (2.0 * np.pi))
MAGIC = float(np.float32(1.5 * 2 ** 23))

M = 256                      # global bins
BPC = M // NCORES            # 32 bins per core
NS = 36                      # build sample batch (35 used: bins+3, padded)
NSU = 35                     # used samples
# PSUM bank = 512 fp32/partition and a matmul output may not cross a bank
# boundary: place 14 NS-wide units per 512-col bank (pad 8 cols), 16 units
# -> 2 banks, so each hidden layer gets its own PSUM bank pair (no reuse).
SLAB = 2 * 512


def ucol(u):
    return 512 * (u // 14) + NS * (u % 14)


# elementwise spans covering exactly the written unit regions (pads excluded).
# The first span holds only units 0,1 (= expert 0) so the next layer's first
# matmuls unblock after a small wrap+sin, not a 476-col one.
SPANS = [(0, 2 * NS), (2 * NS, 7 * NS), (7 * NS, 14 * NS),
         (512, 512 + 2 * NS)]
SPAN_UNITS = [(0, 2), (2, 7), (7, 14), (14, 16)]

# consts tile [128, 64] column layout
C_A0 = 0      # 16: layer-0 scale per unit u=e*2+half
C_C0 = 16     # 16: layer-0 bias
C_BIN = 32    # 1: global bin index per partition (j*32 + p//4, bin-major)
C_XS = 33     # 1: sample-grid position per partition (rows 0..66)
C_GW = 34     # 8: gate_w replicated over sample rows
C_GB = 42     # 8: gate_b replicated
C_BO = 50     # 8: output bias replicated over sample rows
# cf32 = cst(64) | xs(NS) | xb(C2) | crx(4*128 f32, rows 0..66)
CST_W = 64

_BUILD_CACHE: dict = {}
LAST_RESULT = None
DEBUG = False


def _cheb_exp(R):
    """Chebyshev fit of exp on [-R, 0]; ascending coeffs + max abs err."""
    from numpy.polynomial import chebyshev as _C
    from numpy.polynomial import polynomial as _P
    for deg in range(6, 15):
        ch = _C.Chebyshev.interpolate(np.exp, deg, domain=[-R, 0.0])
        p = ch.convert(kind=_P.Polynomial)
        zz = np.linspace(-R, 0, 4001)
        err = float(np.abs(p(zz) - np.exp(zz)).max())
        if err < 5e-5:
            return [float(c) for c in p.coef], err
    return [float(c) for c in p.coef], err


def _build(wrap_twice: bool, C2: int, R: float):
    cheb, _ = _cheb_exp(R)
    deg = len(cheb) - 1
    nc = bacc.Bacc("TRN2", target_bir_lowering=False, debug=False,
                   num_devices=NCORES)

    CW = CST_W + NS + C2
    d_cf32 = nc.dram_tensor("cf32", [128, CW + 512], F32, kind="ExternalInput")
    d_r16 = nc.dram_tensor("r16", [1, NHID * 16 * 128 + 8], F16,
                           kind="ExternalInput")
    d_w16 = nc.dram_tensor("w16", [128, NHID * 4096 + 16], F16,
                           kind="ExternalInput")
    d_out = nc.dram_tensor("out", [128, C2], F32, kind="ExternalOutput")

    with tile.TileContext(nc) as tc:
        with (
            tc.tile_pool(name="cst", bufs=1) as cst_pool,
            tc.tile_pool(name="whp", bufs=1) as wh_pool,
            tc.tile_pool(name="bld", bufs=1) as b_pool,
            tc.tile_pool(name="vwr", bufs=1) as v_pool,
            tc.tile_pool(name="evl", bufs=1) as e_pool,
            tc.tile_pool(name="zpsa", bufs=1, space="PSUM") as z_ps_a,
            tc.tile_pool(name="zpsb", bufs=1, space="PSUM") as z_ps_b,
            tc.tile_pool(name="zpsc", bufs=1, space="PSUM") as z_ps_c,
            tc.tile_pool(name="yps", bufs=1, space="PSUM") as y_ps,
            tc.tile_pool(name="cps", bufs=1, space="PSUM") as c_ps,
        ):
            # ---- input DMAs spread across HWDGE issue queues (SP/Act/DVE)
            # + Pool SWDGE so descriptor issue (650ns each, serial per
            # queue) never gates the copy chain; descriptor-ready order =
            # desired copy order on the shared DMA engines.
            t_cf = cst_pool.tile([128, CW], F32, tag="cf32")
            t_r16 = cst_pool.tile([1, NHID * 16 * 128 + 8], F16, tag="r16")
            t_wh = [wh_pool.tile([128, 4096], F16, tag=f"wh{l}", name=f"wh{l}")
                    for l in range(NHID)]
            t_wo = wh_pool.tile([128, 16], F16, tag="wo")
            t_crx = cst_pool.tile([128, 512], F32, tag="crx")

            def wh_chunk(l, a, b):
                return (t_wh[l][:, a:b], d_w16[:, l * 4096 + a:l * 4096 + b])

            # chunks are expert-aligned (e0-3 | e4-6 | e7); queue slots are
            # chosen so each chunk's descriptor-ready time (SP: 666+650k,
            # Act: 666+657k+134) lands in desired copy order on the FIFO
            # DMA engines.
            # Act gets exactly ONE issue so the Sin act-table load (also on
            # the Act queue) still lands by ~2.6us.
            nc.sync.dma_start(t_cf[:], d_cf32[:, 0:CW])          # SP#1
            nc.scalar.dma_start(*wh_chunk(0, 0, 2048))           # Act#1
            nc.gpsimd.dma_start(t_r16[:], d_r16[:, :])           # Pool#1
            nc.sync.dma_start(*wh_chunk(0, 2048, 3584))          # SP#2
            nc.sync.dma_start(*wh_chunk(1, 0, 2048))             # SP#3
            nc.gpsimd.dma_start(*wh_chunk(0, 3584, 4096))        # Pool#2
            nc.sync.dma_start(*wh_chunk(1, 2048, 3584))          # SP#4
            nc.sync.dma_start(*wh_chunk(1, 3584, 4096))          # SP#5
            nc.sync.dma_start(*wh_chunk(2, 0, 2048))             # SP#6
            nc.sync.dma_start(*wh_chunk(2, 2048, 3584))          # SP#7
            nc.sync.dma_start(*wh_chunk(2, 3584, 4096))          # SP#8
            nc.sync.dma_start(t_wo[:],
                              d_w16[:, NHID * 4096:NHID * 4096 + 16])
            nc.sync.dma_start(t_crx[0:NSU, :], d_cf32[0:NSU, CW:CW + 512])

            t_cst = t_cf[:, 0:CST_W]
            t_xs = t_cf[:, CST_W:CST_W + NS]
            t_xb = t_cf[:, CST_W + NS:CST_W + NS + C2]
            ap_bin = t_cf[:, C_BIN:C_BIN + 1]
            ap_xsc = t_cf[0:NSU, C_XS:C_XS + 1]
            ap_gw = t_cf[0:NSU, C_GW:C_GW + 8]
            ap_gb = t_cf[0:NSU, C_GB:C_GB + 8]
            ap_bo = t_cf[0:NSU, C_BO:C_BO + 8]

            t_on = cst_pool.tile([1, NS], F16, tag="ones")
            nc.gpsimd.memset(t_on[:], 1.0)

            # ---- layer 0 (zb = a*x + c; magic round; ACT Sin).
            # Sin is the only Act function -> its table set loads at t~0.
            # All SBUF staging tiles are PER SPAN: subtile writes into one
            # big tile coarsen to whole-tile WAW deps, which the sem
            # splitter turns into SEQ-blocking EventSemaphores.
            SPAN_W = [hi - lo for lo, hi in SPANS]

            def uspan(u):
                return 0 if u < 2 else 1 if u < 7 else 2 if u < 14 else 3

            def ucs(u):
                return ucol(u) - SPANS[uspan(u)][0]

            t_zb = [b_pool.tile([128, SPAN_W[si]], F32, tag=f"zb{si}",
                                 name=f"zb{si}") for si in range(4)]
            t_k = [b_pool.tile([128, SPAN_W[si]], F32, tag=f"k{si}",
                                name=f"k{si}") for si in range(4)]
            t_v0 = [b_pool.tile([128, SPAN_W[si]], F32, tag=f"v0{si}",
                                 name=f"v0{si}") for si in range(4)]
            t_h = [[b_pool.tile([128, SPAN_W[si]], F16, tag=f"h{l}_{si}",
                                 name=f"h{l}_{si}") for si in range(4)]
                   for l in range(NLAYERS)]

            chain = {"pe": None, "act": None, "dve": None}

            def link(key, first, last=None):
                # Serialize same-engine groups in emission (= need-time)
                # order. The engines are in-order anyway; this pins the
                # scheduler (whose CoreSim model has no DMA bandwidth
                # contention) to the real-timeline-friendly order.
                if chain[key] is not None:
                    add_dep_helper(first.ins, chain[key].ins,
                                   reason="need-order chain")
                chain[key] = last if last is not None else first

            def zb_unit(u):
                eng = nc.vector if u % 2 == 0 else nc.gpsimd
                eng.tensor_scalar(t_zb[uspan(u)][:, ucs(u):ucs(u) + NS], t_xs,
                                  t_cf[:, C_A0 + u:C_A0 + u + 1],
                                  t_cf[:, C_C0 + u:C_C0 + u + 1],
                                  ALU.mult, ALU.add)

            def l0_span(si):
                nc.vector.tensor_scalar(t_k[si][:], t_zb[si][:], MAGIC, MAGIC,
                                        ALU.add, ALU.subtract)
                nc.vector.tensor_tensor(t_v0[si][:], t_zb[si][:], t_k[si][:],
                                        ALU.subtract)
                i_sin = nc.scalar.activation(t_h[0][si][:], t_v0[si][:],
                                             AT.Sin, bias=0.0, scale=TWO_PI)
                link("act", i_sin)

            # span-ordered: units of span si then its round+sin; the gate-exp
            # chain (tiny [67,8] ops) slots in after span 1 so it runs in the
            # weight-DMA shadow without delaying the first sins.
            for si in range(4):
                for u in range(*SPAN_UNITS[si]):
                    zb_unit(u)
                l0_span(si)
                if si == 0:
                    t_z = e_pool.tile([NSU, 8], F32, tag="z")
                    t_s = e_pool.tile([NSU, 1], F32, tag="s")
                    t_z2 = e_pool.tile([NSU, 8], F32, tag="z2")
                    nc.vector.scalar_tensor_tensor(t_z[:], ap_gw, ap_xsc,
                                                   ap_gb, ALU.mult, ALU.add)
                    nc.vector.tensor_reduce(t_s[:], t_z[:], AXL.X, ALU.max)
                    nc.vector.tensor_scalar(t_z2[:], t_z[:], t_s[:],
                                            -float(R), ALU.subtract, ALU.max)
                    # Horner on Pool (gpsimd): u = cheb(z2), abs err < 5e-5
                    t_u = e_pool.tile([NSU, 8], F32, tag="u")
                    t_hz = e_pool.tile([NSU, 8], F32, tag="hz")
                    nc.gpsimd.tensor_scalar(t_u[:], t_z2[:], cheb[deg],
                                            cheb[deg - 1], ALU.mult, ALU.add)
                    for k in range(deg - 2, -1, -1):
                        nc.gpsimd.tensor_tensor(t_hz[:], t_u[:], t_z2[:],
                                                ALU.mult)
                        nc.gpsimd.tensor_scalar_add(t_u[:], t_hz[:], cheb[k])
                    # D table: expert tree-sum of u, kept on Pool so it
                    # queues behind the Horner instead of stalling DVE.
                    t_nd = e_pool.tile([NSU, 2], F32, tag="nd")
                    t_a4 = e_pool.tile([NSU, 4], F32, tag="a4")
                    nc.gpsimd.tensor_tensor(t_a4[:], t_u[:, 0:4],
                                            t_u[:, 4:8], ALU.add)
                    nc.gpsimd.tensor_tensor(t_a4[:, 0:2], t_a4[:, 0:2],
                                            t_a4[:, 2:4], ALU.add)
                    nc.gpsimd.tensor_tensor(t_nd[:, 1:2], t_a4[:, 0:1],
                                            t_a4[:, 1:2], ALU.add)

            # ---- per-point t, t^2 (off critical path, DMA shadow)
            t_t = e_pool.tile([128, C2], F32, tag="t")
            t_t2 = e_pool.tile([128, C2], F32, tag="t2")
            nc.vector.tensor_scalar(t_t[:], t_xb, float(M), ap_bin,
                                    ALU.mult, ALU.subtract)
            nc.gpsimd.tensor_tensor(t_t2[:], t_t[:], t_t[:], ALU.mult)

            # ---- hidden layers: fp16 matmuls (+fp16 K=1 bias matmul),
            # range-reduce on DVE, Sin on Act
            for l in range(1, NLAYERS):
                lw = l - 1
                p_z = (z_ps_a, z_ps_b, z_ps_c)[l - 1].tile(
                    [128, SLAB], F32, tag="z", name=f"z{l}")
                for si in range(4):
                    g_first = g_last = None
                    for u in range(*SPAN_UNITS[si]):
                        e, half = divmod(u, 2)
                        out_sl = p_z[:, ucol(u):ucol(u) + NS]
                        for kc in range(2):
                            wc = ((e * 2 + kc) * 2 + half) * 128
                            uu = e * 2 + kc
                            i_mm = nc.tensor.matmul(
                                out_sl, t_wh[lw][:, wc:wc + 128],
                                t_h[l - 1][uspan(uu)][:, ucs(uu):
                                                      ucs(uu) + NS],
                                start=(kc == 0), stop=False)
                            g_first = g_first or i_mm
                        i_mm = nc.tensor.matmul(
                            out_sl, t_r16[0:1, (lw * 16 + u) * 128:
                                          (lw * 16 + u + 1) * 128],
                            t_on[:], start=False, stop=True)
                        g_last = i_mm
                    link("pe", g_first, g_last)
                    lo, hi = SPANS[si]
                    sl = slice(lo, hi)
                    t_v = v_pool.tile([128, SPAN_W[si]], F32,
                                      tag=f"v{l}_{si}", name=f"v{l}_{si}")
                    if wrap_twice:
                        t_w2 = v_pool.tile([128, SPAN_W[si]], F32,
                                           tag=f"w2_{l}{si}",
                                           name=f"w2_{l}{si}")
                        nc.vector._custom_dve(ADD_RANGE_WRAP, out=t_w2[:],
                                              in0=p_z[:, sl], s0=0.0,
                                              s1=1.0, imm2=2.0)
                        nc.vector._custom_dve(ADD_RANGE_WRAP, out=t_v[:],
                                              in0=t_w2[:], s0=0.0,
                                              s1=0.5, imm2=1.0)
                    else:
                        i_w = nc.vector._custom_dve(ADD_RANGE_WRAP,
                                                    out=t_v[:],
                                                    in0=p_z[:, sl], s0=0.0,
                                                    s1=0.5, imm2=1.0)
                        link("dve", i_w)
                    i_sin = nc.scalar.activation(t_h[l][si][:], t_v[:],
                                                 AT.Sin, bias=0.0,
                                                 scale=TWO_PI)
                    link("act", i_sin)

            # ---- output layer, transposed: p_yT[s, e] via h3-stationary
            # matmuls (Ldweights is free); +bo via a K=1 ones matmul.
            p_yT = y_ps.tile([NSU, 8], F32, tag="yT")
            for e in range(E):
                g_first = g_last = None
                for kc in range(2):
                    u = e * 2 + kc
                    i_mm = nc.tensor.matmul(
                        p_yT[:, e:e + 1],
                        t_h[NLAYERS - 1][uspan(u)][:, ucs(u):ucs(u) + NSU],
                        t_wo[:, u:u + 1],
                        start=(kc == 0), stop=(kc == 1),
                        skip_group_check=True)
                    g_first = g_first or i_mm
                    g_last = i_mm
                link("pe", g_first, g_last)

            # ---- N table + Catmull-Rom coefficients via PE
            # (bo is folded in on DVE: et = (yT + bo) * u)
            t_et = e_pool.tile([NSU, 8], F32, tag="et")
            t_yb = e_pool.tile([NSU, 8], F32, tag="yb")
            nc.vector.tensor_tensor(t_yb[:], p_yT[:], ap_bo, ALU.add)
            nc.vector.scalar_tensor_tensor(t_et[:], t_yb[:], 1.0, t_u[:],
                                           ALU.mult, ALU.mult,
                                           accum_out=t_nd[:, 0:1])
            p_coef = c_ps.tile([128, 8], F32, tag="coef")
            g_first = g_last = None
            for X in range(4):
                for f in range(2):
                    i_mm = nc.tensor.matmul(
                        p_coef[:, X + 4 * f:X + 4 * f + 1],
                        t_crx[0:NSU, X * 128:(X + 1) * 128],
                        t_nd[:, f:f + 1], start=True, stop=True)
                    g_first = g_first or i_mm
                    g_last = i_mm
            link("pe", g_first, g_last)

            # ---- per-point cubics: NH = (cA t + cB) t^2 + (cC t + cD),
            # same for DH; out = NH/DH. All on DVE (coef scalars straight
            # from PSUM; GPSIMD can't read PSUM).
            cN = [p_coef[:, i:i + 1] for i in range(4)]
            cD = [p_coef[:, 4 + i:5 + i] for i in range(4)]
            t_w1 = e_pool.tile([128, C2], F32, tag="w1")
            t_w2e = e_pool.tile([128, C2], F32, tag="w2e")
            t_w3 = e_pool.tile([128, C2], F32, tag="w3")
            t_vD = e_pool.tile([128, C2], F32, tag="vD")
            t_u1 = e_pool.tile([128, C2], F32, tag="u1")
            t_u2 = e_pool.tile([128, C2], F32, tag="u2")
            t_u3 = e_pool.tile([128, C2], F32, tag="u3")
            t_vN = e_pool.tile([128, C2], F32, tag="vN")
            t_rcp = e_pool.tile([128, C2], F32, tag="rcp")
            t_outm = e_pool.tile([128, C2], F32, tag="outm")
            nc.vector.tensor_scalar(t_w1[:], t_t[:], cD[0], cD[1],
                                    ALU.mult, ALU.add)
            nc.vector.tensor_scalar(t_w2e[:], t_t[:], cD[2], cD[3],
                                    ALU.mult, ALU.add)
            nc.vector.tensor_tensor(t_w3[:], t_w1[:], t_t2[:], ALU.mult)
            nc.vector.tensor_tensor(t_vD[:], t_w3[:], t_w2e[:], ALU.add)
            nc.vector.reciprocal(t_rcp[:], t_vD[:])
            nc.vector.tensor_scalar(t_u1[:], t_t[:], cN[0], cN[1],
                                    ALU.mult, ALU.add)
            nc.vector.tensor_scalar(t_u2[:], t_t[:], cN[2], cN[3],
                                    ALU.mult, ALU.add)
            nc.gpsimd.tensor_tensor(t_u3[:], t_u1[:], t_t2[:], ALU.mult)
            nc.vector.tensor_tensor(t_vN[:], t_u3[:], t_u2[:], ALU.add)
            nc.vector.tensor_tensor(t_outm[:], t_vN[:], t_rcp[:], ALU.mult)
            nc.sync.dma_start(d_out[:, :], t_outm[:])
            if DEBUG:
                d_dbg = nc.dram_tensor("dbg", [128, 48], F32,
                                       kind="ExternalOutput")
                nc.sync.dma_start(d_dbg[0:67, 0:8], t_u[:])
                nc.sync.dma_start(d_dbg[0:67, 8:10], t_nd[:])
                nc.sync.dma_start(d_dbg[:, 16:24], t_coef[:])
                nc.sync.dma_start(d_dbg[:, 24:32], t_t[:, 0:8])
                nc.sync.dma_start(d_dbg[0:67, 32:40], t_et[:])
                nc.sync.dma_start(d_dbg[0:67, 40:48], t_z[:])
                d_yT = nc.dram_tensor("dbg_yT", [67, 8], F32,
                                      kind="ExternalOutput")
                t_yTc = e_pool.tile([67, 8], F32, tag="yTc")
                nc.vector.tensor_scalar(t_yTc[:], p_yT[:], 1.0, None,
                                        ALU.mult)
                nc.sync.dma_start(d_yT[:, :], t_yTc[:])

    nc.compile()
    return nc


def _host_pack(x, gate_w, gate_b, w0, b0, wh, bh, wo, bo, C2):
    """Host: range-shard + bin points, pack weights/consts. Index math only."""
    xf = x.reshape(-1)
    NP = xf.size
    gbin = np.clip((xf.astype(np.float64) * M).astype(np.int64), 0, M - 1)
    core = gbin >> 5
    binlo = gbin & (BPC - 1)

    counts = np.zeros((NCORES, BPC), np.int64)
    slot = np.empty(NP, np.int64)
    for i in range(NP):
        c, b = core[i], binlo[i]
        slot[i] = counts[c, b]
        counts[c, b] += 1
    maxc = int(counts.max())
    if maxc > 4 * C2:
        return None, None, maxc

    placement = (core, binlo, slot)

    # fp16 weight pack (shared across cores): wh blocks | wo columns
    w16 = np.zeros((128, NHID * 4096 + 16), np.float16)
    for l in range(NHID):
        for e in range(E):
            for kc in range(2):
                for mc in range(2):
                    colbase = l * 4096 + ((e * 2 + kc) * 2 + mc) * 128
                    blk = (SC * wh[l, e, mc * 128:(mc + 1) * 128,
                                   kc * 128:(kc + 1) * 128]).T
                    w16[:, colbase:colbase + 128] = blk.astype(np.float16)
    for u in range(16):
        e, half = divmod(u, 2)
        w16[:, NHID * 4096 + u] = \
            wo[e, 0, half * 128:(half + 1) * 128].astype(np.float16)

    r16 = np.zeros((1, NHID * 16 * 128 + 8), np.float16)
    for l in range(NHID):
        for u in range(16):
            e, half = divmod(u, 2)
            r16[0, (l * 16 + u) * 128:(l * 16 + u + 1) * 128] = \
                (SC * bh[l, e, half * 128:(half + 1) * 128]).astype(np.float16)
    r16[0, NHID * 16 * 128:] = bo[:, 0].astype(np.float16)

    # Catmull-Rom shift matrices, f32: crx[s, X*128+p], bin-major
    # partitions p = 4*bin + q; nonzero at s=b..b+3.
    CRW = np.array([[-0.5, 1.5, -1.5, 0.5],
                    [1.0, -2.5, 2.0, -0.5],
                    [-0.5, 0.0, 0.5, 0.0],
                    [0.0, 1.0, 0.0, 0.0]], np.float64)
    crx = np.zeros((128, 512), np.float32)
    for p in range(128):
        b = p // 4
        for X in range(4):
            for k in range(4):
                crx[b + k, X * 128 + p] = CRW[X, k]

    cst_base = np.zeros((128, CST_W), np.float32)
    for u in range(16):
        e, half = divmod(u, 2)
        cst_base[:, C_A0 + u] = SC * w0[e, half * 128:(half + 1) * 128, 0]
        cst_base[:, C_C0 + u] = SC * b0[e, half * 128:(half + 1) * 128]
    cst_base[0:NSU, C_GW:C_GW + 8] = gate_w[:, 0][None, :]
    cst_base[0:NSU, C_GB:C_GB + 8] = gate_b[None, :]
    cst_base[0:NSU, C_BO:C_BO + 8] = bo[:, 0][None, :]

    CW = CST_W + NS + C2
    in_maps = []
    for j in range(NCORES):
        cst = cst_base.copy()
        cst[:, C_BIN] = j * BPC + (np.arange(128) >> 2)
        gi = j * BPC - 1 + np.arange(NSU)
        cst[0:NSU, C_XS] = (gi / M).astype(np.float32)
        # binned x: partition p = 4*bin + q, C2 slots per quarter
        xbj = np.empty((128, C2), np.float32)
        for b in range(BPC):
            fill = np.full(4 * C2, (j * BPC + b + 0.5) / M, np.float32)
            sel = (core == j) & (binlo == b)
            vals = xf[sel]
            fill[:vals.size] = vals
            for q in range(4):
                xbj[4 * b + q, :] = fill[q * C2:(q + 1) * C2]
        xsg = j * BPC - 1 + np.minimum(np.arange(NS), BPC + 2)
        assert NSU == BPC + 3
        cf32 = np.empty((128, CW + 512), np.float32)
        cf32[:, 0:CST_W] = cst
        cf32[:, CST_W:CST_W + NS] = \
            np.broadcast_to((xsg / M).astype(np.float32), (128, NS))
        cf32[:, CST_W + NS:CW] = xbj
        cf32[:, CW:] = crx
        in_maps.append({"cf32": cf32, "r16": r16, "w16": w16})
    return in_maps, placement, maxc


def kernel(x, gate_w, gate_b, w0, b0, wh, bh, wo, bo):
    x = np.asarray(x, dtype=np.float32)
    gate_w = np.asarray(gate_w, dtype=np.float32)
    gate_b = np.asarray(gate_b, dtype=np.float32)
    w0 = np.asarray(w0, dtype=np.float32)
    b0 = np.asarray(b0, dtype=np.float32)
    wh = np.asarray(wh, dtype=np.float32)
    bh = np.asarray(bh, dtype=np.float32)
    wo = np.asarray(wo, dtype=np.float32)
    bo = np.asarray(bo, dtype=np.float32)

    # hidden pre-activation range (turns) over the union of build grids
    gi = np.arange(-1, M + NS - BPC + 2, dtype=np.float64) / M
    xs = gi.astype(np.float32)
    a = (SC * w0[:, :, 0]).astype(np.float32)
    c = (SC * b0).astype(np.float32)
    zb = a[:, :, None] * xs[None, None, :] + c[:, :, None]
    h = np.sin(TWO_PI * (zb - np.round(zb))).astype(np.float32)
    hid_bound = 0.0
    for l in range(NHID):
        z = (np.einsum('egh,eht->egt', (SC * wh[l]).astype(np.float32), h)
             + (SC * bh[l]).astype(np.float32)[:, :, None])
        hid_bound = max(hid_bound, float(np.abs(z).max()))
        h = np.sin(TWO_PI * (z - np.round(z))).astype(np.float32)
    hid_bound *= 1.02
    assert hid_bound < 2.90, f"hidden range {hid_bound} needs >2 wraps"
    wrap_twice = bool(hid_bound >= 1.45)

    # gate logit spread on the grid -> exp poly range R
    zg = gate_w[:, 0][None, :] * xs[:, None] + gate_b[None, :]
    spread = float((zg.max(1) - zg.min(1)).max())
    R = float(max(2.0, np.ceil(spread + 0.3)))

    C2 = 44
    while True:
        in_maps, placement, maxc = _host_pack(
            x, gate_w, gate_b, w0, b0, wh, bh, wo, bo, C2)
        if in_maps is not None:
            break
        C2 = ((maxc + 3) // 4 + 3) // 4 * 4

    key = (wrap_twice, C2, R)
    if key not in _BUILD_CACHE:
        _BUILD_CACHE[key] = _build(wrap_twice, C2, R)
    nc = _BUILD_CACHE[key]

    global LAST_RESULT
    LAST_RESULT = run_bass_kernel_spmd(nc, in_maps, list(range(NCORES)))
    res = LAST_RESULT.results

    core, binlo, slot = placement
    out = np.empty(x.size, np.float32)
    for j in range(NCORES):
        oj = res[j]["out"]            # [128, C2]
        sel = np.nonzero(core == j)[0]
        sl = slot[sel]
        out[sel] = oj[4 * binlo[sel] + sl // C2, sl % C2]
    return out.reshape(B, N, 1).astype(np.float32)


# revision 32
# speedup vs baseline: 1.1958x; 1.0285x over previous
"""MoE-SIREN (nn_MoE_36146444763329) Trainium2 Bass kernel, v3: N/D tables.

The whole MoE output is a univariate function of x:
    f(x) = N(x)/D(x),  N = sum_e exp(z_e(x)) y_e(x),  D = sum_e exp(z_e(x)),
with z_e = gate logits. Each core (range-sharded points, 64 global bins):
  1. builds the 8 expert SIREN tables over ITS x-range on device (fp16
     matmuls, fp32 range-reduction, ACT Sin; Sin is the ONLY Act function
     so its table set loads once at t=0),
  2. evaluates the gate exp on the sample grid with a Chebyshev polynomial
     on DVE/Pool (no Act Exp -> no second act-table load),
  3. forms the transposed tables y^T[sample, expert] directly via PE
     matmuls with h3 as the stationary operand (Ldweights is free),
  4. reduces N/D over experts (DVE free-dim reduce) and turns the tables
     into per-bin Catmull-Rom coefficients with host-baked shift x CR
     matrices as PE matmuls -> coefficients land as per-partition scalars,
  5. evaluates two cubic Horner chains + one divide per point.

No per-point exp, no moment products, no mid-kernel SBUF->SBUF coef DMA
(each DMA costs ~2.2us fixed latency in HWDGE+DGE+sem-prop). Weight DMAs
are chunked so each layer's matmuls start as soon as its bytes land.

Points layout per core: partition p = bin + 64*half, C2 slots per half
(C2 grown if an input overflows; seed-0 max occupancy is 86 -> C2=44).
Host binning is index computation only; all value math runs on device.
"""

import numpy as np

import concourse.bass as bass
import concourse.mybir as mybir
import concourse.tile as tile
from concourse import bacc
from concourse.bass_utils import run_bass_kernel_spmd
from concourse.dve_ops import ADD_RANGE_WRAP
from concourse.tile_rust import add_dep_helper

F32 = mybir.dt.float32
F16 = mybir.dt.float16
AT = mybir.ActivationFunctionType
ALU = mybir.AluOpType
AXL = mybir.AxisListType

B, N, E, H, NLAYERS = 2, 16384, 8, 256, 4
OMEGA0 = 30.0
NCORES = 8
NHID = NLAYERS - 1
TWO_PI = float(2.0 * np.pi)
SC = float(OMEGA0 / (2.0 * np.pi))
MAGIC = float(np.float32(1.5 * 2 ** 23))

M = 256                      # global bins
BPC = M // NCORES            # 32 bins per core
NS = 36                      # build sample batch (35 used: bins+3, padded)
NSU = 35                     # used samples
# PSUM bank = 512 fp32/partition and a matmul output may not cross a bank
# boundary: place 14 NS-wide units per 512-col bank (pad 8 cols), 16 units
# -> 2 banks, so each hidden layer gets its own PSUM bank pair (no reuse).
SLAB = 2 * 512


def ucol(u):
    return 512 * (u // 14) + NS * (u % 14)


# elementwise spans covering exactly the written unit regions (pads excluded).
# The first span holds only units 0,1 (= expert 0) so the next layer's first
# matmuls unblock after a small wrap+sin, not a 476-col one.
SPANS = [(0, 2 * NS), (2 * NS, 7 * NS), (7 * NS, 14 * NS),
         (512, 512 + 2 * NS)]
SPAN_UNITS = [(0, 2), (2, 7), (7, 14), (14, 16)]

# consts tile [128, 64] column layout
C_A0 = 0      # 16: layer-0 scale per unit u=e*2+half
C_C0 = 16     # 16: layer-0 bias
C_BIN = 32    # 1: global bin index per partition (j*32 + p//4, bin-major)
C_XS = 33     # 1: sample-grid position per partition (rows 0..66)
C_GW = 34     # 8: gate_w replicated over sample rows
C_GB = 42     # 8: gate_b replicated
C_BO = 50     # 8: output bias replicated over sample rows
# cf32 = cst(64) | xs(NS) | xb(C2) | crx(4*128 f32, rows 0..66)
CST_W = 64

_BUILD_CACHE: dict = {}
LAST_RESULT = None
DEBUG = False


def _cheb_exp(R):
    """Chebyshev fit of exp on [-R, 0]; ascending coeffs + max abs err."""
    from numpy.polynomial import chebyshev as _C
    from numpy.polynomial import polynomial as _P
    for deg in range(6, 15):
        ch = _C.Chebyshev.interpolate(np.exp, deg, domain=[-R, 0.0])
        p = ch.convert(kind=_P.Polynomial)
        zz = np.linspace(-R, 0, 4001)
        err = float(np.abs(p(zz) - np.exp(zz)).max())
        if err < 5e-5:
            return [float(c) for c in p.coef], err
    return [float(c) for c in p.coef], err


def _build(wrap_twice: bool, C2: int, R: float):
    cheb, _ = _cheb_exp(R)
    deg = len(cheb) - 1
    nc = bacc.Bacc("TRN2", target_bir_lowering=False, debug=False,
                   num_devices=NCORES)

    CW = CST_W + NS + C2
    d_cf32 = nc.dram_tensor("cf32", [128, CW + 512], F32, kind="ExternalInput")
    d_r16 = nc.dram_tensor("r16", [1, NHID * 16 * 128 + 8], F16,
                           kind="ExternalInput")
    d_w16 = nc.dram_tensor("w16", [128, NHID * 4096 + 16], F16,
                           kind="ExternalInput")
    d_out = nc.dram_tensor("out", [128, C2], F32, kind="ExternalOutput")

    with tile.TileContext(nc) as tc:
        with (
            tc.tile_pool(name="cst", bufs=1) as cst_pool,
            tc.tile_pool(name="whp", bufs=1) as wh_pool,
            tc.tile_pool(name="bld", bufs=1) as b_pool,
            tc.tile_pool(name="vwr", bufs=1) as v_pool,
            tc.tile_pool(name="evl", bufs=1) as e_pool,
            tc.tile_pool(name="zpsa", bufs=1, space="PSUM") as z_ps_a,
            tc.tile_pool(name="zpsb", bufs=1, space="PSUM") as z_ps_b,
            tc.tile_pool(name="zpsc", bufs=1, space="PSUM") as z_ps_c,
            tc.tile_pool(name="yps", bufs=1, space="PSUM") as y_ps,
            tc.tile_pool(name="cps", bufs=1, space="PSUM") as c_ps,
        ):
            # ---- input DMAs spread across HWDGE issue queues (SP/Act/DVE)
            # + Pool SWDGE so descriptor issue (650ns each, serial per
            # queue) never gates the copy chain; descriptor-ready order =
            # desired copy order on the shared DMA engines.
            t_cf = cst_pool.tile([128, CW], F32, tag="cf32")
            t_r16 = cst_pool.tile([1, NHID * 16 * 128 + 8], F16, tag="r16")
            t_wh = [wh_pool.tile([128, 4096], F16, tag=f"wh{l}", name=f"wh{l}")
                    for l in range(NHID)]
            t_wo = wh_pool.tile([128, 16], F16, tag="wo")
            t_crx = cst_pool.tile([128, 512], F32, tag="crx")

            def wh_chunk(l, a, b):
                return (t_wh[l][:, a:b], d_w16[:, l * 4096 + a:l * 4096 + b])

            # chunks are expert-aligned (e0-3 | e4-6 | e7); queue slots are
            # chosen so each chunk's descriptor-ready time (SP: 666+650k,
            # Act: 666+657k+134) lands in desired copy order on the FIFO
            # DMA engines.
            # Act gets exactly ONE issue so the Sin act-table load (also on
            # the Act queue) still lands by ~2.6us.
            nc.sync.dma_start(t_cf[:], d_cf32[:, 0:CW])          # SP#1
            nc.scalar.dma_start(*wh_chunk(0, 0, 2048))           # Act#1
            nc.gpsimd.dma_start(t_r16[:], d_r16[:, :])           # Pool#1
            nc.sync.dma_start(*wh_chunk(0, 2048, 3584))          # SP#2
            nc.sync.dma_start(*wh_chunk(1, 0, 2048))             # SP#3
            nc.gpsimd.dma_start(*wh_chunk(0, 3584, 4096))        # Pool#2
            nc.sync.dma_start(*wh_chunk(1, 2048, 3584))          # SP#4
            nc.sync.dma_start(*wh_chunk(1, 3584, 4096))          # SP#5
            nc.sync.dma_start(*wh_chunk(2, 0, 2048))             # SP#6
            nc.sync.dma_start(*wh_chunk(2, 2048, 3584))          # SP#7
            nc.sync.dma_start(*wh_chunk(2, 3584, 4096))          # SP#8
            nc.sync.dma_start(t_wo[:],
                              d_w16[:, NHID * 4096:NHID * 4096 + 16])
            nc.sync.dma_start(t_crx[0:NSU, :], d_cf32[0:NSU, CW:CW + 512])

            t_cst = t_cf[:, 0:CST_W]
            t_xs = t_cf[:, CST_W:CST_W + NS]
            t_xb = t_cf[:, CST_W + NS:CST_W + NS + C2]
            ap_bin = t_cf[:, C_BIN:C_BIN + 1]
            ap_xsc = t_cf[0:NSU, C_XS:C_XS + 1]
            ap_gw = t_cf[0:NSU, C_GW:C_GW + 8]
            ap_gb = t_cf[0:NSU, C_GB:C_GB + 8]
            ap_bo = t_cf[0:NSU, C_BO:C_BO + 8]

            t_on = cst_pool.tile([1, NS], F16, tag="ones")
            nc.gpsimd.memset(t_on[:], 1.0)

            # ---- layer 0 (zb = a*x + c; magic round; ACT Sin).
            # Sin is the only Act function -> its table set loads at t~0.
            # All SBUF staging tiles are PER SPAN: subtile writes into one
            # big tile coarsen to whole-tile WAW deps, which the sem
            # splitter turns into SEQ-blocking EventSemaphores.
            SPAN_W = [hi - lo for lo, hi in SPANS]

            def uspan(u):
                return 0 if u < 2 else 1 if u < 7 else 2 if u < 14 else 3

            def ucs(u):
                return ucol(u) - SPANS[uspan(u)][0]

            t_zb = [b_pool.tile([128, SPAN_W[si]], F32, tag=f"zb{si}",
                                 name=f"zb{si}") for si in range(4)]
            t_k = [b_pool.tile([128, SPAN_W[si]], F32, tag=f"k{si}",
                                name=f"k{si}") for si in range(4)]
            t_v0 = [b_pool.tile([128, SPAN_W[si]], F32, tag=f"v0{si}",
                                 name=f"v0{si}") for si in range(4)]
            t_h = [[b_pool.tile([128, SPAN_W[si]], F16, tag=f"h{l}_{si}",
                                 name=f"h{l}_{si}") for si in range(4)]
                   for l in range(NLAYERS)]

            chain = {"pe": None, "act": None, "dve": None}

            def link(key, first, last=None):
                # Serialize same-engine groups in emission (= need-time)
                # order. The engines are in-order anyway; this pins the
                # scheduler (whose CoreSim model has no DMA bandwidth
                # contention) to the real-timeline-friendly order.
                if key != "pe":
                    return
                if chain[key] is not None:
                    add_dep_helper(first.ins, chain[key].ins,
                                   reason="need-order chain")
                chain[key] = last if last is not None else first

            def zb_unit(u):
                eng = nc.vector if u % 2 == 0 else nc.gpsimd
                eng.tensor_scalar(t_zb[uspan(u)][:, ucs(u):ucs(u) + NS], t_xs,
                                  t_cf[:, C_A0 + u:C_A0 + u + 1],
                                  t_cf[:, C_C0 + u:C_C0 + u + 1],
                                  ALU.mult, ALU.add)

            def l0_span(si):
                nc.vector.tensor_scalar(t_k[si][:], t_zb[si][:], MAGIC, MAGIC,
                                        ALU.add, ALU.subtract)
                nc.vector.tensor_tensor(t_v0[si][:], t_zb[si][:], t_k[si][:],
                                        ALU.subtract)
                i_sin = nc.scalar.activation(t_h[0][si][:], t_v0[si][:],
                                             AT.Sin, bias=0.0, scale=TWO_PI)
                link("act", i_sin)

            # span-ordered: units of span si then its round+sin; the gate-exp
            # chain (tiny [67,8] ops) slots in after span 1 so it runs in the
            # weight-DMA shadow without delaying the first sins.
            for si in range(4):
                for u in range(*SPAN_UNITS[si]):
                    zb_unit(u)
                l0_span(si)
                if si == 0:
                    t_z = e_pool.tile([NSU, 8], F32, tag="z")
                    t_s = e_pool.tile([NSU, 1], F32, tag="s")
                    t_z2 = e_pool.tile([NSU, 8], F32, tag="z2")
                    nc.vector.scalar_tensor_tensor(t_z[:], ap_gw, ap_xsc,
                                                   ap_gb, ALU.mult, ALU.add)
                    nc.vector.tensor_reduce(t_s[:], t_z[:], AXL.X, ALU.max)
                    nc.vector.tensor_scalar(t_z2[:], t_z[:], t_s[:],
                                            -float(R), ALU.subtract, ALU.max)
                    # Horner on Pool (gpsimd): u = cheb(z2), abs err < 5e-5
                    t_u = e_pool.tile([NSU, 8], F32, tag="u")
                    t_hz = e_pool.tile([NSU, 8], F32, tag="hz")
                    nc.gpsimd.tensor_scalar(t_u[:], t_z2[:], cheb[deg],
                                            cheb[deg - 1], ALU.mult, ALU.add)
                    for k in range(deg - 2, -1, -1):
                        nc.gpsimd.tensor_tensor(t_hz[:], t_u[:], t_z2[:],
                                                ALU.mult)
                        nc.gpsimd.tensor_scalar_add(t_u[:], t_hz[:], cheb[k])
                    # D table: expert tree-sum of u, kept on Pool so it
                    # queues behind the Horner instead of stalling DVE.
                    t_nd = e_pool.tile([NSU, 2], F32, tag="nd")
                    t_a4 = e_pool.tile([NSU, 4], F32, tag="a4")
                    nc.gpsimd.tensor_tensor(t_a4[:], t_u[:, 0:4],
                                            t_u[:, 4:8], ALU.add)
                    nc.gpsimd.tensor_tensor(t_a4[:, 0:2], t_a4[:, 0:2],
                                            t_a4[:, 2:4], ALU.add)
                    nc.gpsimd.tensor_tensor(t_nd[:, 1:2], t_a4[:, 0:1],
                                            t_a4[:, 1:2], ALU.add)

            # ---- per-point t, t^2 (off critical path, DMA shadow)
            t_t = e_pool.tile([128, C2], F32, tag="t")
            t_t2 = e_pool.tile([128, C2], F32, tag="t2")
            nc.vector.tensor_scalar(t_t[:], t_xb, float(M), ap_bin,
                                    ALU.mult, ALU.subtract)
            nc.gpsimd.tensor_tensor(t_t2[:], t_t[:], t_t[:], ALU.mult)

            # ---- hidden layers: fp16 matmuls (+fp16 K=1 bias matmul),
            # range-reduce on DVE, Sin on Act
            for l in range(1, NLAYERS):
                lw = l - 1
                p_z = (z_ps_a, z_ps_b, z_ps_c)[l - 1].tile(
                    [128, SLAB], F32, tag="z", name=f"z{l}")
                for si in range(4):
                    g_first = g_last = None
                    for u in range(*SPAN_UNITS[si]):
                        e, half = divmod(u, 2)
                        out_sl = p_z[:, ucol(u):ucol(u) + NS]
                        for kc in range(2):
                            wc = ((e * 2 + kc) * 2 + half) * 128
                            uu = e * 2 + kc
                            i_mm = nc.tensor.matmul(
                                out_sl, t_wh[lw][:, wc:wc + 128],
                                t_h[l - 1][uspan(uu)][:, ucs(uu):
                                                      ucs(uu) + NS],
                                start=(kc == 0), stop=False)
                            g_first = g_first or i_mm
                        i_mm = nc.tensor.matmul(
                            out_sl, t_r16[0:1, (lw * 16 + u) * 128:
                                          (lw * 16 + u + 1) * 128],
                            t_on[:], start=False, stop=True)
                        g_last = i_mm
                    link("pe", g_first, g_last)
                    lo, hi = SPANS[si]
                    sl = slice(lo, hi)
                    t_v = v_pool.tile([128, SPAN_W[si]], F32,
                                      tag=f"v{l}_{si}", name=f"v{l}_{si}")
                    if wrap_twice:
                        t_w2 = v_pool.tile([128, SPAN_W[si]], F32,
                                           tag=f"w2_{l}{si}",
                                           name=f"w2_{l}{si}")
                        nc.vector._custom_dve(ADD_RANGE_WRAP, out=t_w2[:],
                                              in0=p_z[:, sl], s0=0.0,
                                              s1=1.0, imm2=2.0)
                        nc.vector._custom_dve(ADD_RANGE_WRAP, out=t_v[:],
                                              in0=t_w2[:], s0=0.0,
                                              s1=0.5, imm2=1.0)
                    else:
                        i_w = nc.vector._custom_dve(ADD_RANGE_WRAP,
                                                    out=t_v[:],
                                                    in0=p_z[:, sl], s0=0.0,
                                                    s1=0.5, imm2=1.0)
                        link("dve", i_w)
                    i_sin = nc.scalar.activation(t_h[l][si][:], t_v[:],
                                                 AT.Sin, bias=0.0,
                                                 scale=TWO_PI)
                    link("act", i_sin)

            # ---- output layer, transposed: p_yT[s, e] via h3-stationary
            # matmuls (Ldweights is free); +bo via a K=1 ones matmul.
            p_yT = y_ps.tile([NSU, 8], F32, tag="yT")
            for e in range(E):
                g_first = g_last = None
                for kc in range(2):
                    u = e * 2 + kc
                    i_mm = nc.tensor.matmul(
                        p_yT[:, e:e + 1],
                        t_h[NLAYERS - 1][uspan(u)][:, ucs(u):ucs(u) + NSU],
                        t_wo[:, u:u + 1],
                        start=(kc == 0), stop=(kc == 1),
                        skip_group_check=True)
                    g_first = g_first or i_mm
                    g_last = i_mm
                link("pe", g_first, g_last)

            # ---- N table + Catmull-Rom coefficients via PE
            # (bo is folded in on DVE: et = (yT + bo) * u)
            t_et = e_pool.tile([NSU, 8], F32, tag="et")
            t_yb = e_pool.tile([NSU, 8], F32, tag="yb")
            nc.vector.tensor_tensor(t_yb[:], p_yT[:], ap_bo, ALU.add)
            nc.vector.scalar_tensor_tensor(t_et[:], t_yb[:], 1.0, t_u[:],
                                           ALU.mult, ALU.mult,
                                           accum_out=t_nd[:, 0:1])
            p_coef = c_ps.tile([128, 8], F32, tag="coef")
            g_first = g_last = None
            for X in range(4):
                for f in range(2):
                    i_mm = nc.tensor.matmul(
                        p_coef[:, X + 4 * f:X + 4 * f + 1],
                        t_crx[0:NSU, X * 128:(X + 1) * 128],
                        t_nd[:, f:f + 1], start=True, stop=True)
                    g_first = g_first or i_mm
                    g_last = i_mm
            link("pe", g_first, g_last)

            # ---- per-point cubics: NH = (cA t + cB) t^2 + (cC t + cD),
            # same for DH; out = NH/DH. All on DVE (coef scalars straight
            # from PSUM; GPSIMD can't read PSUM).
            cN = [p_coef[:, i:i + 1] for i in range(4)]
            cD = [p_coef[:, 4 + i:5 + i] for i in range(4)]
            t_w1 = e_pool.tile([128, C2], F32, tag="w1")
            t_w2e = e_pool.tile([128, C2], F32, tag="w2e")
            t_w3 = e_pool.tile([128, C2], F32, tag="w3")
            t_vD = e_pool.tile([128, C2], F32, tag="vD")
            t_u1 = e_pool.tile([128, C2], F32, tag="u1")
            t_u2 = e_pool.tile([128, C2], F32, tag="u2")
            t_u3 = e_pool.tile([128, C2], F32, tag="u3")
            t_vN = e_pool.tile([128, C2], F32, tag="vN")
            t_rcp = e_pool.tile([128, C2], F32, tag="rcp")
            t_outm = e_pool.tile([128, C2], F32, tag="outm")
            nc.vector.tensor_scalar(t_w1[:], t_t[:], cD[0], cD[1],
                                    ALU.mult, ALU.add)
            nc.vector.tensor_scalar(t_w2e[:], t_t[:], cD[2], cD[3],
                                    ALU.mult, ALU.add)
            nc.vector.tensor_tensor(t_w3[:], t_w1[:], t_t2[:], ALU.mult)
            nc.vector.tensor_tensor(t_vD[:], t_w3[:], t_w2e[:], ALU.add)
            nc.vector.reciprocal(t_rcp[:], t_vD[:])
            nc.vector.tensor_scalar(t_u1[:], t_t[:], cN[0], cN[1],
                                    ALU.mult, ALU.add)
            nc.vector.tensor_scalar(t_u2[:], t_t[:], cN[2], cN[3],
                                    ALU.mult, ALU.add)
            nc.gpsimd.tensor_tensor(t_u3[:], t_u1[:], t_t2[:], ALU.mult)
            nc.vector.tensor_tensor(t_vN[:], t_u3[:], t_u2[:], ALU.add)
            nc.vector.tensor_tensor(t_outm[:], t_vN[:], t_rcp[:], ALU.mult)
            nc.sync.dma_start(d_out[:, :], t_outm[:])
            if DEBUG:
                d_dbg = nc.dram_tensor("dbg", [128, 48], F32,
                                       kind="ExternalOutput")
                nc.sync.dma_start(d_dbg[0:67, 0:8], t_u[:])
                nc.sync.dma_start(d_dbg[0:67, 8:10], t_nd[:])
                nc.sync.dma_start(d_dbg[:, 16:24], t_coef[:])
                nc.sync.dma_start(d_dbg[:, 24:32], t_t[:, 0:8])
                nc.sync.dma_start(d_dbg[0:67, 32:40], t_et[:])
                nc.sync.dma_start(d_dbg[0:67, 40:48], t_z[:])
                d_yT = nc.dram_tensor("dbg_yT", [67, 8], F32,
                                      kind="ExternalOutput")
                t_yTc = e_pool.tile([67, 8], F32, tag="yTc")
                nc.vector.tensor_scalar(t_yTc[:], p_yT[:], 1.0, None,
                                        ALU.mult)
                nc.sync.dma_start(d_yT[:, :], t_yTc[:])

    nc.compile()
    return nc


def _host_pack(x, gate_w, gate_b, w0, b0, wh, bh, wo, bo, C2):
    """Host: range-shard + bin points, pack weights/consts. Index math only."""
    xf = x.reshape(-1)
    NP = xf.size
    gbin = np.clip((xf.astype(np.float64) * M).astype(np.int64), 0, M - 1)
    core = gbin >> 5
    binlo = gbin & (BPC - 1)

    counts = np.zeros((NCORES, BPC), np.int64)
    slot = np.empty(NP, np.int64)
    for i in range(NP):
        c, b = core[i], binlo[i]
        slot[i] = counts[c, b]
        counts[c, b] += 1
    maxc = int(counts.max())
    if maxc > 4 * C2:
        return None, None, maxc

    placement = (core, binlo, slot)

    # fp16 weight pack (shared across cores): wh blocks | wo columns
    w16 = np.zeros((128, NHID * 4096 + 16), np.float16)
    for l in range(NHID):
        for e in range(E):
            for kc in range(2):
                for mc in range(2):
                    colbase = l * 4096 + ((e * 2 + kc) * 2 + mc) * 128
                    blk = (SC * wh[l, e, mc * 128:(mc + 1) * 128,
                                   kc * 128:(kc + 1) * 128]).T
                    w16[:, colbase:colbase + 128] = blk.astype(np.float16)
    for u in range(16):
        e, half = divmod(u, 2)
        w16[:, NHID * 4096 + u] = \
            wo[e, 0, half * 128:(half + 1) * 128].astype(np.float16)

    r16 = np.zeros((1, NHID * 16 * 128 + 8), np.float16)
    for l in range(NHID):
        for u in range(16):
            e, half = divmod(u, 2)
            r16[0, (l * 16 + u) * 128:(l * 16 + u + 1) * 128] = \
                (SC * bh[l, e, half * 128:(half + 1) * 128]).astype(np.float16)
    r16[0, NHID * 16 * 128:] = bo[:, 0].astype(np.float16)

    # Catmull-Rom shift matrices, f32: crx[s, X*128+p], bin-major
    # partitions p = 4*bin + q; nonzero at s=b..b+3.
    CRW = np.array([[-0.5, 1.5, -1.5, 0.5],
                    [1.0, -2.5, 2.0, -0.5],
                    [-0.5, 0.0, 0.5, 0.0],
                    [0.0, 1.0, 0.0, 0.0]], np.float64)
    crx = np.zeros((128, 512), np.float32)
    for p in range(128):
        b = p // 4
        for X in range(4):
            for k in range(4):
                crx[b + k, X * 128 + p] = CRW[X, k]

    cst_base = np.zeros((128, CST_W), np.float32)
    for u in range(16):
        e, half = divmod(u, 2)
        cst_base[:, C_A0 + u] = SC * w0[e, half * 128:(half + 1) * 128, 0]
        cst_base[:, C_C0 + u] = SC * b0[e, half * 128:(half + 1) * 128]
    cst_base[0:NSU, C_GW:C_GW + 8] = gate_w[:, 0][None, :]
    cst_base[0:NSU, C_GB:C_GB + 8] = gate_b[None, :]
    cst_base[0:NSU, C_BO:C_BO + 8] = bo[:, 0][None, :]

    CW = CST_W + NS + C2
    in_maps = []
    for j in range(NCORES):
        cst = cst_base.copy()
        cst[:, C_BIN] = j * BPC + (np.arange(128) >> 2)
        gi = j * BPC - 1 + np.arange(NSU)
        cst[0:NSU, C_XS] = (gi / M).astype(np.float32)
        # binned x: partition p = 4*bin + q, C2 slots per quarter
        xbj = np.empty((128, C2), np.float32)
        for b in range(BPC):
            fill = np.full(4 * C2, (j * BPC + b + 0.5) / M, np.float32)
            sel = (core == j) & (binlo == b)
            vals = xf[sel]
            fill[:vals.size] = vals
            for q in range(4):
                xbj[4 * b + q, :] = fill[q * C2:(q + 1) * C2]
        xsg = j * BPC - 1 + np.minimum(np.arange(NS), BPC + 2)
        assert NSU == BPC + 3
        cf32 = np.empty((128, CW + 512), np.float32)
        cf32[:, 0:CST_W] = cst
        cf32[:, CST_W:CST_W + NS] = \
            np.broadcast_to((xsg / M).astype(np.float32), (128, NS))
        cf32[:, CST_W + NS:CW] = xbj
        cf32[:, CW:] = crx
        in_maps.append({"cf32": cf32, "r16": r16, "w16": w16})
    return in_maps, placement, maxc


def kernel(x, gate_w, gate_b, w0, b0, wh, bh, wo, bo):
    x = np.asarray(x, dtype=np.float32)
    gate_w = np.asarray(gate_w, dtype=np.float32)
    gate_b = np.asarray(gate_b, dtype=np.float32)
    w0 = np.asarray(w0, dtype=np.float32)
    b0 = np.asarray(b0, dtype=np.float32)
    wh = np.asarray(wh, dtype=np.float32)
    bh = np.asarray(bh, dtype=np.float32)
    wo = np.asarray(wo, dtype=np.float32)
    bo = np.asarray(bo, dtype=np.float32)

    # hidden pre-activation range (turns) over the union of build grids
    gi = np.arange(-1, M + NS - BPC + 2, dtype=np.float64) / M
    xs = gi.astype(np.float32)
    a = (SC * w0[:, :, 0]).astype(np.float32)
    c = (SC * b0).astype(np.float32)
    zb = a[:, :, None] * xs[None, None, :] + c[:, :, None]
    h = np.sin(TWO_PI * (zb - np.round(zb))).astype(np.float32)
    hid_bound = 0.0
    for l in range(NHID):
        z = (np.einsum('egh,eht->egt', (SC * wh[l]).astype(np.float32), h)
             + (SC * bh[l]).astype(np.float32)[:, :, None])
        hid_bound = max(hid_bound, float(np.abs(z).max()))
        h = np.sin(TWO_PI * (z - np.round(z))).astype(np.float32)
    hid_bound *= 1.02
    assert hid_bound < 2.90, f"hidden range {hid_bound} needs >2 wraps"
    wrap_twice = bool(hid_bound >= 1.45)

    # gate logit spread on the grid -> exp poly range R
    zg = gate_w[:, 0][None, :] * xs[:, None] + gate_b[None, :]
    spread = float((zg.max(1) - zg.min(1)).max())
    R = float(max(2.0, np.ceil(spread + 0.3)))

    C2 = 44
    while True:
        in_maps, placement, maxc = _host_pack(
            x, gate_w, gate_b, w0, b0, wh, bh, wo, bo, C2)
        if in_maps is not None:
            break
        C2 = ((maxc + 3) // 4 + 3) // 4 * 4

    key = (wrap_twice, C2, R)
    if key not in _BUILD_CACHE:
        _BUILD_CACHE[key] = _build(wrap_twice, C2, R)
    nc = _BUILD_CACHE[key]

    global LAST_RESULT
    LAST_RESULT = run_bass_kernel_spmd(nc, in_maps, list(range(NCORES)))
    res = LAST_RESULT.results

    core, binlo, slot = placement
    out = np.empty(x.size, np.float32)
    for j in range(NCORES):
        oj = res[j]["out"]            # [128, C2]
        sel = np.nonzero(core == j)[0]
        sl = slot[sel]
        out[sel] = oj[4 * binlo[sel] + sl // C2, sl % C2]
    return out.reshape(B, N, 1).astype(np.float32)
